# revision 19
# baseline (speedup 1.0000x reference)
"""Trainium2 Bass kernel for the two-direction masked cross-attention module.

Math (per batch b):
  qk_a = amr @ W_amr_qk, qk_v = vis @ W_vis_qk, v_a = amr @ W_amr_v, v_v = vis @ W_vis_v
  S[h,n,m] = qk_a[h,n,:] . qk_v[h,m,:] / sqrt(hd)
  out_amr = (softmax_m(S) @ v_v  merged) @ W_amr_out + b_amr_out, elementwise * amr
  out_vis = vis * ((softmax_n(S^T masked) @ v_a merged) @ W_vis_out + b_vis_out)

Key facts used:
  - scores are bounded (|S| < ~10 for this data distribution), so softmax can
    skip the max-subtraction: softmax = exp(S) / sum(exp(S)).  The mask on the
    n axis is applied multiplicatively to v_a rows and to the denominator
    ones-vector instead of as -inf score offsets (exp(-inf) = 0 equivalent).
  - The two directions need exp(S) in transposed layouts ([m,n] vs [n,m]);
    cross-partition transposes are expensive, so each direction computes its
    own scores+exp (K=32 matmuls are cheap; ACT exp is the bottleneck).

Sharding: 8 cores = (batch b in 0..3) x (half s in 0..1).  Core (b,s) computes
  - amr-direction output for amr rows [s*256, s*256+256)  (full m)
  - vis-direction output for vis rows [s*2048, s*2048+2048)  (full n)
One single SPMD program: the host feeds each core its batch's tensors ROLLED so
that the core's half comes first; attention sums are permutation-invariant in
the key axis, so the roll is transparent to the math.
"""

import os
import sys

import numpy as np

for _p in ("/opt/trn_rl_repo",):
    if os.path.isdir(_p) and _p not in sys.path:
        sys.path.append(_p)

import concourse.bass as bass  # noqa: E402
import concourse.mybir as mybir  # noqa: E402
import concourse.tile as tile  # noqa: E402
from concourse import bacc  # noqa: E402
from concourse.bass_utils import run_bass_kernel_spmd  # noqa: E402
from concourse.masks import make_identity  # noqa: E402


# PSUM accumulation with col-tiled (tile_position) streams sharing one bank:
# HW-verified (microtest.py): each stream keeps its OWN start/stop group —
# start=True clears/overwrites only the stream's partition rows.  The sim's
# zero-region checker can't track partition-sliced groups, so those matmuls
# pass skip_group_check=True.

F32 = mybir.dt.float32
F32R = mybir.dt.float32r
AF = mybir.ActivationFunctionType

B, N, M, D, H, HD = 4, 512, 4096, 256, 8, 32
NQ = N // 2  # 256 amr query rows per core
MQ = M // 2  # 2048 vis query rows per core
SCALE = float(1.0 / np.sqrt(HD))

# float32r runs the PE at 1 cycle/row (vs 4 for float32) for moving dim >= 256.
USE_F32R = False

# debug: 0 = loads+transposes+projections only, 1 = +phase A, 2 = full
K_PHASES = int(os.environ.get("K_PHASES", "2"))
# debug sub-bisect of phase A: 1 = scores+exp only, 2 = +numer/denom acc,
# 3 = +recip/mul, 4 = full (out-proj)
K_SUB = int(os.environ.get("K_SUB", "4"))


def _mm(ap):
    return ap.bitcast(F32R) if USE_F32R else ap


def build_nc():
    nc = bacc.Bacc("TRN2", target_bir_lowering=False, debug=False)

    amr_d = nc.dram_tensor("amr", [N, D], F32, kind="ExternalInput")
    vis_d = nc.dram_tensor("vis", [M, D], F32, kind="ExternalInput")
    keep_d = nc.dram_tensor("keep", [N], F32, kind="ExternalInput")
    w_names = ["w_aqk", "w_vqk", "w_av", "w_vv", "w_ao", "w_vo"]
    w_d = {n: nc.dram_tensor(n, [D, D], F32, kind="ExternalInput") for n in w_names}
    b_ao_d = nc.dram_tensor("b_ao", [D], F32, kind="ExternalInput")
    b_vo_d = nc.dram_tensor("b_vo", [D], F32, kind="ExternalInput")
    out_amr_d = nc.dram_tensor("out_amr", [NQ, D], F32, kind="ExternalOutput")
    out_vis_d = nc.dram_tensor("out_vis", [MQ, D], F32, kind="ExternalOutput")

    with tile.TileContext(nc) as tc:
        from contextlib import ExitStack

        with ExitStack() as ctx:
            const = ctx.enter_context(tc.tile_pool(name="const", bufs=1))
            persist = ctx.enter_context(tc.tile_pool(name="persist", bufs=1))

            # ---- constants ----
            identity = const.tile([128, 128], F32)
            make_identity(nc, identity)
            ones32 = const.tile([128, 32], F32)
            nc.vector.memset(ones32, 1.0)
            onesrow = const.tile([1, 128], F32)
            nc.vector.memset(onesrow, 1.0)
            bias_ao = const.tile([1, D], F32)
            nc.sync.dma_start(out=bias_ao, in_=b_ao_d.ap()[None, :])
            bias_vo = const.tile([1, D], F32)
            nc.sync.dma_start(out=bias_vo, in_=b_vo_d.ap()[None, :])
            keep_sb = const.tile([128, 4], F32)
            nc.sync.dma_start(
                out=keep_sb, in_=keep_d.ap().rearrange("(nb p) -> p nb", p=128)
            )
            keep32 = const.tile([128, 4, 32], F32)
            for nb in range(4):
                nc.vector.tensor_scalar_mul(keep32[:, nb, :], ones32, keep_sb[:, nb : nb + 1])

            # ---- input loads ----
            amr_sb = persist.tile([128, 4, D], F32)  # [p, nb, d]
            nc.sync.dma_start(
                out=amr_sb, in_=amr_d.ap().rearrange("(nb p) d -> p nb d", p=128)
            )
            vis_sb = persist.tile([128, 16, D], F32)  # first half (the Q half)
            for g in range(4):
                nc.sync.dma_start(
                    out=vis_sb[:, g * 4 : (g + 1) * 4, :],
                    in_=vis_d.ap()[g * 512 : (g + 1) * 512, :].rearrange(
                        "(mb p) d -> p mb d", p=128
                    ),
                )
            w_sb = {}
            for n in w_names:
                t = persist.tile([128, 2, D], F32, tag=f"w_{n}")
                nc.sync.dma_start(
                    out=t, in_=w_d[n].ap().rearrange(" (kc p) d -> p kc d", p=128)
                )
                w_sb[n] = t

            amrT = persist.tile([128, 2, N], F32)  # [d_p, dc, n]
            amr_qkT = persist.tile([128, 2, N], F32)  # [c_p, cc, n]
            vis_qkT = persist.tile([128, 2, M], F32)  # [c_p, cc, m]
            vis_v = persist.tile([128, 32, D], F32)  # [m_p, mb, c]
            va = persist.tile([128, 4, D], F32)  # [n_p, nb, c] (mask-scaled)

            # ---- phase 1: transpose inputs to [d, tokens] ----
            with tc.tile_pool(name="visT", bufs=1) as vistp, tc.tile_pool(
                name="tpsum", bufs=2, space="PSUM"
            ) as tpsum, tc.tile_pool(name="ld2", bufs=4) as ld2:
                visualT = vistp.tile([128, 2, M], F32)  # [d_p, dc, m]
                for dc in range(2):
                    tp = tpsum.tile([128, 512], F32)
                    for nb in range(4):
                        nc.tensor.transpose(
                            tp[:, nb * 128 : (nb + 1) * 128],
                            amr_sb[:, nb, dc * 128 : (dc + 1) * 128],
                            identity,
                        )
                    nc.vector.tensor_copy(amrT[:, dc, :], tp)
                for mg in range(8):
                    if mg < 4:
                        srcs = [vis_sb[:, mg * 4 + k, :] for k in range(4)]
                    else:
                        srcs = []
                        for k in range(4):
                            mb = mg * 4 + k
                            t = ld2.tile([128, D], F32)
                            nc.sync.dma_start(
                                out=t, in_=vis_d.ap()[mb * 128 : (mb + 1) * 128, :]
                            )
                            srcs.append(t)
                    for dc in range(2):
                        tp = tpsum.tile([128, 512], F32)
                        for k in range(4):
                            nc.tensor.transpose(
                                tp[:, k * 128 : (k + 1) * 128],
                                srcs[k][:, dc * 128 : (dc + 1) * 128],
                                identity,
                            )
                        nc.vector.tensor_copy(
                            visualT[:, dc, mg * 512 : (mg + 1) * 512], tp
                        )

                # ---- phase 2: projections ----
                with tc.tile_pool(name="p2psum", bufs=4, space="PSUM") as p2psum:
                    for cc in range(2):
                        pp = p2psum.tile([128, 512], F32)
                        for dc in range(2):
                            nc.tensor.matmul(
                                pp,
                                _mm(w_sb["w_aqk"][:, dc, cc * 128 : (cc + 1) * 128]),
                                _mm(amrT[:, dc, :]),
                                start=dc == 0,
                                stop=dc == 1,
                            )
                        nc.vector.tensor_copy(amr_qkT[:, cc, :], pp)
                    for cc in range(2):
                        for mc8 in range(8):
                            pp = p2psum.tile([128, 512], F32)
                            for dc in range(2):
                                nc.tensor.matmul(
                                    pp,
                                    _mm(w_sb["w_vqk"][:, dc, cc * 128 : (cc + 1) * 128]),
                                    _mm(visualT[:, dc, mc8 * 512 : (mc8 + 1) * 512]),
                                    start=dc == 0,
                                    stop=dc == 1,
                                )
                            nc.scalar.copy(
                                vis_qkT[:, cc, mc8 * 512 : (mc8 + 1) * 512], pp
                            )
                    for mb in range(32):
                        pp = p2psum.tile([128, 512], F32)
                        for dc in range(2):
                            nc.tensor.matmul(
                                pp[:, :D],
                                _mm(visualT[:, dc, mb * 128 : (mb + 1) * 128]),
                                _mm(w_sb["w_vv"][:, dc, :]),
                                start=dc == 0,
                                stop=dc == 1,
                            )
                        nc.vector.tensor_copy(vis_v[:, mb, :], pp[:, :D])
                    for nb in range(4):
                        pp = p2psum.tile([128, 512], F32)
                        for dc in range(2):
                            nc.tensor.matmul(
                                pp[:, :D],
                                _mm(amrT[:, dc, nb * 128 : (nb + 1) * 128]),
                                _mm(w_sb["w_av"][:, dc, :]),
                                start=dc == 0,
                                stop=dc == 1,
                            )
                        nc.vector.tensor_scalar_mul(
                            va[:, nb, :], pp[:, :D], keep_sb[:, nb : nb + 1]
                        )

            # debug early-exit: dump junk into outputs so the NEFF is complete
            if K_PHASES < 1:
                with tc.tile_pool(name="dbg", bufs=2) as dbg:
                    for nb2 in range(2):
                        ot = dbg.tile([128, D], F32)
                        nc.vector.tensor_copy(ot, va[:, nb2, :])
                        nc.sync.dma_start(out=out_amr_d.ap()[nb2 * 128 : (nb2 + 1) * 128, :], in_=ot)
                    for mb in range(16):
                        ot = dbg.tile([128, D], F32)
                        nc.vector.tensor_copy(ot, vis_v[:, mb, :])
                        nc.sync.dma_start(out=out_vis_d.ap()[mb * 128 : (mb + 1) * 128, :], in_=ot)

            # ---- phase A: amr -> vis direction (scores transposed: [m, n]) ----
            attn_a = []
            if K_PHASES >= 1:
              with tc.tile_pool(name="attn_a", bufs=1) as apool_sb:
                with tc.tile_pool(name="apsum", bufs=2, space="PSUM") as apool, tc.tile_pool(
                    name="stp", bufs=2, space="PSUM"
                ) as stpool, tc.tile_pool(name="epool", bufs=3) as epool:
                    for hg in range(2):
                        acc_n = apool.tile([128, 256], F32, tag="acc_n")
                        acc_d = apool.tile([128, 256], F32, tag="acc_d")
                        # score packs: 2 heads x 2 m-blocks per [128,1024] tile.
                        # Concurrent row-tiled matmuls (different heads) land in
                        # different PSUM banks; same-bank pairs share a row
                        # group, so their drains are sequential (HW-verified
                        # constraint: concurrent matmuls must not share a
                        # (partition, bank) region).
                        for mp in range(16):
                            for hp in range(2):
                                stp = stpool.tile([128, 1024], F32)
                                for mbb in range(2):
                                    for hh2 in range(2):
                                        hh = hp * 2 + hh2
                                        mb = mp * 2 + mbb
                                        off = hh2 * 512 + mbb * 256
                                        nc.tensor.matmul(
                                            stp[:, off : off + 256],
                                            _mm(vis_qkT[32 * hh : 32 * hh + 32, hg, mb * 128 : (mb + 1) * 128]),
                                            _mm(amr_qkT[32 * hh : 32 * hh + 32, hg, 0:NQ]),
                                            start=True,
                                            stop=True,
                                            tile_position=(32 * hh, 0),
                                        )
                                e = epool.tile([128, 1024], F32)
                                nc.scalar.activation(e, stp, AF.Exp, scale=SCALE)
                                if K_SUB < 2:
                                    dsb = epool.tile([128, 1024], F32, tag="dump")
                                    nc.vector.tensor_copy(dsb, e)
                                    continue
                                for mbb in range(2):
                                    for hh2 in range(2):
                                        hh = hp * 2 + hh2
                                        h = hg * 4 + hh
                                        mb = mp * 2 + mbb
                                        off = hh2 * 512 + mbb * 256
                                        nc.tensor.matmul(
                                            acc_n[32 * hh : 32 * hh + 32, :],
                                            _mm(vis_v[:, mb, 32 * h : 32 * h + 32]),
                                            _mm(e[:, off : off + 256]),
                                            start=mb == 0,
                                            stop=mb == 31,
                                            tile_position=(0, 32 * hh),
                                            skip_group_check=True,
                                        )
                                        nc.tensor.matmul(
                                            acc_d[32 * hh : 32 * hh + 32, :],
                                            _mm(ones32),
                                            _mm(e[:, off : off + 256]),
                                            start=mb == 0,
                                            stop=mb == 31,
                                            tile_position=(0, 32 * hh),
                                            skip_group_check=True,
                                        )
                        if K_SUB < 3:
                            at = apool_sb.tile([128, 256], F32, tag=f"attn_a{hg}")
                            nc.vector.tensor_copy(at, acc_n if K_SUB == 2 else va[:, 0, :])
                            attn_a.append(at)
                            continue
                        rec = epool.tile([128, 256], F32, tag="rec_a")
                        nc.vector.reciprocal(rec, acc_d)
                        at = apool_sb.tile([128, 256], F32, tag=f"attn_a{hg}")
                        nc.vector.tensor_mul(at, acc_n, rec)
                        attn_a.append(at)

                # amr out-projection + bias + elementwise
                with tc.tile_pool(name="aops", bufs=2, space="PSUM") as aops, tc.tile_pool(
                    name="aosb", bufs=2
                ) as aosb:
                    for nb2 in range(2):
                        if K_SUB < 4:
                            ot = aosb.tile([128, D], F32)
                            if K_SUB < 2:
                                nc.vector.tensor_copy(ot, amr_sb[:, nb2, :])
                            else:
                                nc.vector.tensor_copy(ot, attn_a[0])
                            nc.sync.dma_start(out=out_amr_d.ap()[nb2 * 128 : (nb2 + 1) * 128, :], in_=ot)
                            continue
                        po = aops.tile([128, D], F32)
                        for hg in range(2):
                            nc.tensor.matmul(
                                po,
                                _mm(attn_a[hg][:, nb2 * 128 : (nb2 + 1) * 128]),
                                _mm(w_sb["w_ao"][:, hg, :]),
                                start=hg == 0,
                                stop=False,
                            )
                        nc.tensor.matmul(
                            po, _mm(onesrow), _mm(bias_ao), start=False, stop=True
                        )
                        ot = aosb.tile([128, D], F32)
                        nc.vector.tensor_mul(ot, po, amr_sb[:, nb2, :])
                        nc.sync.dma_start(
                            out=out_amr_d.ap()[nb2 * 128 : (nb2 + 1) * 128, :], in_=ot
                        )

            # ---- phase V: vis -> amr direction (scores natural: [n, m]) ----
            if K_PHASES < 2:
                with tc.tile_pool(name="dbg2", bufs=2) as dbg2:
                    for mb in range(16):
                        ot = dbg2.tile([128, D], F32)
                        nc.vector.tensor_copy(ot, vis_v[:, mb, :])
                        nc.sync.dma_start(out=out_vis_d.ap()[mb * 128 : (mb + 1) * 128, :], in_=ot)
            if K_PHASES >= 2:
              with tc.tile_pool(name="attnv", bufs=1) as attnvp:
                attn_v = attnvp.tile([128, 2, MQ], F32)  # [c_p, hg, m]
                with tc.tile_pool(name="vacc", bufs=2, space="PSUM") as vaccp, tc.tile_pool(
                    name="svp", bufs=2, space="PSUM"
                ) as svp, tc.tile_pool(name="evpool", bufs=3) as evpool:
                    for hg in range(2):
                        for mc in range(4):
                            accv_n = vaccp.tile([128, 512], F32, tag="accv_n")
                            accv_d = vaccp.tile([128, 512], F32, tag="accv_d")
                            for nb in range(4):
                                es = []
                                for hp in range(2):
                                    sv = svp.tile([128, 1024], F32)
                                    for hh2 in range(2):
                                        hh = hp * 2 + hh2
                                        nc.tensor.matmul(
                                            sv[:, hh2 * 512 : (hh2 + 1) * 512],
                                            _mm(amr_qkT[32 * hh : 32 * hh + 32, hg, nb * 128 : (nb + 1) * 128]),
                                            _mm(vis_qkT[32 * hh : 32 * hh + 32, hg, mc * 512 : (mc + 1) * 512]),
                                            start=True,
                                            stop=True,
                                            tile_position=(32 * hh, 0),
                                        )
                                    ev = evpool.tile([128, 1024], F32)
                                    nc.scalar.activation(ev, sv, AF.Exp, scale=SCALE)
                                    es.append(ev)
                                for hp in range(2):
                                    for hh2 in range(2):
                                        hh = hp * 2 + hh2
                                        h = hg * 4 + hh
                                        nc.tensor.matmul(
                                            accv_n[32 * hh : 32 * hh + 32, :],
                                            _mm(va[:, nb, 32 * h : 32 * h + 32]),
                                            _mm(es[hp][:, hh2 * 512 : (hh2 + 1) * 512]),
                                            start=nb == 0,
                                            stop=nb == 3,
                                            tile_position=(0, 32 * hh),
                                            skip_group_check=True,
                                        )
                                        nc.tensor.matmul(
                                            accv_d[32 * hh : 32 * hh + 32, :],
                                            _mm(keep32[:, nb, :]),
                                            _mm(es[hp][:, hh2 * 512 : (hh2 + 1) * 512]),
                                            start=nb == 0,
                                            stop=nb == 3,
                                            tile_position=(0, 32 * hh),
                                            skip_group_check=True,
                                        )
                            recv = evpool.tile([128, 512], F32, tag="rec_v")
                            nc.vector.reciprocal(recv, accv_d)
                            nc.vector.tensor_mul(
                                attn_v[:, hg, mc * 512 : (mc + 1) * 512],
                                accv_n,
                                recv,
                            )

                # vis out-projection + bias + elementwise
                with tc.tile_pool(name="vops", bufs=2, space="PSUM") as vops, tc.tile_pool(
                    name="vosb", bufs=3
                ) as vosb:
                    for mb in range(16):
                        po = vops.tile([128, D], F32)
                        for hg in range(2):
                            nc.tensor.matmul(
                                po,
                                _mm(attn_v[:, hg, mb * 128 : (mb + 1) * 128]),
                                _mm(w_sb["w_vo"][:, hg, :]),
                                start=hg == 0,
                                stop=False,
                            )
                        nc.tensor.matmul(
                            po, _mm(onesrow), _mm(bias_vo), start=False, stop=True
                        )
                        ot = vosb.tile([128, D], F32)
                        nc.vector.tensor_mul(ot, po, vis_sb[:, mb, :])
                        nc.sync.dma_start(
                            out=out_vis_d.ap()[mb * 128 : (mb + 1) * 128, :], in_=ot
                        )

    nc.compile()
    return nc


_NC = None


def _get_nc():
    global _NC
    if _NC is None:
        _NC = build_nc()
    return _NC


def run(inputs, trace=False):
    amr = np.ascontiguousarray(np.asarray(inputs["amr_feats"], dtype=np.float32))
    vis = np.ascontiguousarray(np.asarray(inputs["visual_feats"], dtype=np.float32))
    mask = np.asarray(inputs["amr_pad_mask"])
    keep = np.ascontiguousarray((~mask).astype(np.float32))
    w = {
        "w_aqk": inputs["W_amr_qk"],
        "w_vqk": inputs["W_vis_qk"],
        "w_av": inputs["W_amr_v"],
        "w_vv": inputs["W_vis_v"],
        "w_ao": inputs["W_amr_out"],
        "w_vo": inputs["W_vis_out"],
        "b_ao": inputs["b_amr_out"],
        "b_vo": inputs["b_vis_out"],
    }
    w = {k: np.ascontiguousarray(np.asarray(v, dtype=np.float32)) for k, v in w.items()}

    in_maps = []
    for c in range(8):
        b, s = c // 2, c % 2
        in_maps.append(
            dict(
                amr=np.ascontiguousarray(np.roll(amr[b], -s * NQ, axis=0)),
                vis=np.ascontiguousarray(np.roll(vis[b], -s * MQ, axis=0)),
                keep=np.ascontiguousarray(np.roll(keep[b], -s * NQ)),
                **w,
            )
        )

    nc = _get_nc()
    kwargs = {}
    if trace:
        kwargs = dict(trace=True, trace_cores=list(range(8)))
    res = run_bass_kernel_spmd(nc, in_maps, core_ids=list(range(8)), **kwargs)

    out_amr = np.empty((B, N, D), np.float32)
    out_vis = np.empty((B, M, D), np.float32)
    for c in range(8):
        b, s = c // 2, c % 2
        out_amr[b, s * NQ : (s + 1) * NQ] = res.results[c]["out_amr"]
        out_vis[b, s * MQ : (s + 1) * MQ] = res.results[c]["out_vis"]
    return (out_amr, out_vis), res


def kernel(**inputs):
    (out_amr, out_vis), _ = run(inputs)
    return out_amr, out_vis
